# revision 34
# baseline (speedup 1.0000x reference)
"""Trainium2 Bass kernel for nn_EternalNeuralLayer.

Math: out = tanh(x @ W_c + b_c + probs[None, :]) where
probs[j] = |state[j, 0]|^2 after 27 nearest-neighbour circulant "gates"
applied to the uniform state 1/sqrt(n). Each gate matrix
G = cos*I - sin*P + sin*P^T is circulant, and the uniform vector is its
eigenvector with eigenvalue cos(theta), so the state stays uniform:
probs[j] = (prod_{d,g} cos(ew[d, j, g]))^2 / n   (g in 0..2, d in 0..8).

Sharding: data-parallel over the batch (8 cores x 512 rows). Every core
streams the full classical_weights [2048, 2048] and computes its
x-shard's GEMM as outT[m, b] = sum_k W[k, m] * xT[k, b] (output m on
partitions so the per-output bias (b_c + probs) is a per-partition ACT
bias), applies tanh on the Scalar engine directly out of PSUM, and
writes its outT shard. The eternal-probs product is computed on-device
per core from the [27, 2048] angle slice (tiny). No collectives.

GEMM precision: main pass xh @ Wh in float32r (fp32 with 11 explicit
mantissa bits, full PE rate, operands pre-rounded host-side).
Rounding-residual corrections run as wide fp8e5 (e5m2) DoubleRow
matmuls: one instruction computes two independent K=128 plane products
over all 512 out cols (rhs free 1024; the hw accepts >512 moving for
fp8) in 512 cycles -- 2x the fp32r MAC rate per k-tile (measured; the
cost model's 0.5 cyc/row does not materialize on TRN2 silicon in any
layout or perf mode). e5m2's 2^-15 dynamic range holds the
~2^-12-scale residuals unscaled, so corrections accumulate into the
SAME PSUM bank as the main pass (the first DR opens the group via the
2 KB pending-zero region; the last main closes it) and the epilogue
stays one fused tanh, stored as bf16 (tanh is in [-1,1]; bf16 adds
<= 2^-9; host upconverts). Correction coverage is partial (see UNITS):
x-residual on all 16 k-tiles, W-residual on 8. Measured absmax
1.52e-2 vs the 2e-2 gate (3.1e-3 at full coverage); the error is
fully deterministic -- the device result matches the host numpy model
of the exact rounding chain to 5 digits.

PE work per core: 16 m-tiles x (12 DoubleRow + 16 fp32r) matmuls x
512 cycles ~= 98 us at 2.4 GHz, vs 164 us for the previous 3-pass
fp32r hi/lo scheme.

Schedule: two global phases with a PSUM->SBUF spill between them.
Phase 1 runs ALL 16 tiles' fp8 DR correction stages (gated only on
the 6.5 MB fp8 stream -> PE-bound almost immediately); each tile's
partial sum is copied to SBUF by the otherwise-idle Vector engine,
freeing its PSUM bank (8 banks rotate over 32 accumulation groups).
Phase 2 runs each tile's 16 fp32r mains into a fresh bank -- by then
the whole 21 MB fp32r stream has landed behind phase 1, so every
phase-2 tile executes at the 3.63 us floor with zero gaps -- then DVE
adds the spilled correction in place on PSUM and ACT applies the
fused tanh+bias. The first 4 tiles of phase 1 are emitted as an
anti-diagonal wavefront over (tile, x8-chunk) to pace with the fp8
head; the rest run as straight per-tile blocks (tile-hopping costs
~23 ns/matmul in wait decode). All loads ride the single sync HWDGE
ring in consumption order (one ring saturates the ~343 GB/s core
bandwidth; FIFO order = precise arrival control). Output stores use
the scalar ring so a store waiting on ACT never head-of-line-blocks
loads.
"""

import math
import os
import sys

import numpy as np
import ml_dtypes

for _p in ("/opt/trn_rl_repo", "/root/.axon_site/_ro/trn_rl_repo"):
    if _p not in sys.path and os.path.isdir(_p):
        sys.path.append(_p)

import concourse.bass as bass  # noqa: E402
import concourse.tile as tile  # noqa: E402
from concourse import bacc, mybir  # noqa: E402
from concourse.bass_utils import run_bass_kernel_spmd  # noqa: E402

N_CORES = 8
B, N, M, D = 4096, 2048, 2048, 9
BS = B // N_CORES  # 512 batch rows per core
KT = N // 128  # 16 contraction tiles
MT = M // 128  # 16 output m-tiles
MG = 2  # m-tiles per output DMA group
NGATE = D * 3  # 27 rotation gates
GPAD = 32  # padded gate slots (pad with 0.0 -> cos = 1)

# Correction coverage. Each DoubleRow "unit" holds two K=128 plane
# products. ("hl", kb) pairs the x-residual with the W-residual for
# k-tile kb: Wh8[kb]*xl8[kb] + Wl8[kb]*xh8[kb]. ("xx", kb) packs two
# k-tiles of the x-residual correction only: Wh8[kb]*xl8[kb] +
# Wh8[kb+1]*xl8[kb+1]. With WCOV=8 the W-residual is corrected on
# 8/16 k-tiles: measured absmax 1.52e-2 vs the 2e-2 gate (vs 3.1e-3
# at full coverage), for 4 fewer 512-cycle PE instructions per m-tile.
# The error is fully deterministic (device result matches the host
# numpy model of the exact rounding chain to 5 digits).
WCOV = 8
UNITS = [("hl", kb) for kb in range(WCOV)] + [
    ("xx", kb) for kb in range(WCOV, KT, 2)
]
NU = len(UNITS)  # 12

F32 = mybir.dt.float32
F32R = mybir.dt.float32r
F8 = mybir.dt.float8e5
BF16 = mybir.dt.bfloat16
DR = mybir.MatmulPerfMode.DoubleRow


def build_program():
    nc = bacc.Bacc(
        "TRN2", target_bir_lowering=False, debug=False, num_devices=N_CORES
    )
    # xt[p, kb*BS + b] = xh[b, kb*128 + p]  (fp32r high part of x)
    xt_d = nc.dram_tensor("xt", [128, KT * BS], F32R, kind="ExternalInput").ap()
    # x8[p, u, pl, b]: correction-unit moving planes (see UNITS)
    x8_d = nc.dram_tensor("x8", [128, NU, 2, BS], F8, kind="ExternalInput").ap()
    # w[t*128 + p, kb*128 + m] = Wh[kb*128 + p, t*128 + m]  (fp32r)
    w_d = nc.dram_tensor("w", [M, N], F32R, kind="ExternalInput").ap()
    # w8[t*128+p, u, pl, m]: correction-unit stationary planes
    w8_d = nc.dram_tensor("w8", [M, NU, 2, 128], F8, kind="ExternalInput").ap()
    ang_d = nc.dram_tensor("ang", [128, GPAD * MT], F32, kind="ExternalInput").ap()
    cbt_d = nc.dram_tensor("cbt", [128, MT], F32, kind="ExternalInput").ap()
    # out_dev[g, ml, j*BS + b] = tanh(...)[m = (g*MG+j)*128 + ml, b]
    # bf16: tanh output is in [-1, 1], so bf16 adds <= 2^-9 abs error and
    # halves the store traffic; host_post upconverts to fp32.
    out_d = nc.dram_tensor(
        "out_dev", [MT // MG, 128, MG * BS], BF16, kind="ExternalOutput"
    ).ap()

    with tile.TileContext(nc) as tc:
        with (
            tc.tile_pool(name="xt", bufs=1) as xt_pool,
            tc.tile_pool(name="x8", bufs=1) as x8_pool,
            tc.tile_pool(name="w", bufs=6) as w_pool,
            tc.tile_pool(name="w8", bufs=MT) as w8_pool,
            tc.tile_pool(name="ps", bufs=8, space="PSUM") as ps_pool,
            tc.tile_pool(name="out", bufs=3) as out_pool,
            tc.tile_pool(name="spill", bufs=MT) as spill_pool,
            tc.tile_pool(name="small", bufs=1) as small_pool,
        ):
            # --- GEMM input DMAs, all on the sync HWDGE ring so arrival
            # ORDER is exactly the issue order (one ring saturates the
            # ~343 GB/s core bandwidth by itself). Head order feeds the
            # stream-split ramp: fp8 w8/x8 first (corrections run first),
            # then fp32r w/xt. Output stores use the scalar ring so a
            # store waiting on ACT never head-of-line-blocks loads. ---
            wts = {}
            w8ts = {}

            def fetch_w(t):
                wt = w_pool.tile([128, KT * 128], F32R, tag="w")
                nc.sync.dma_start(wt[:], w_d[t * 128 : (t + 1) * 128, :])
                wts[t] = wt

            def fetch_w8(t):
                wt8 = w8_pool.tile([128, NU, 2, 128], F8, tag="w8")
                nc.sync.dma_start(wt8[:], w8_d[t * 128 : (t + 1) * 128])
                w8ts[t] = wt8

            # single xt tile: matmuls read slices of ONE tile object,
            # avoiding the ~20 ns/matmul tile-hop wait-decode cost that
            # separate per-slab tiles incur
            xt_t = xt_pool.tile([128, KT, BS], F32R, name="xt_t")

            def fetch_xt(s):
                nc.sync.dma_start(xt_t[:, s], xt_d[:, s * BS : (s + 1) * BS])

            # one fp8 x tile; fetched in 2-unit chunks
            NCH = (NU + 1) // 2  # 7 chunks of up to 2 units
            x8t = x8_pool.tile([128, NU, 2, BS], F8, name="x8t")

            def fetch_x8(c):
                nc.sync.dma_start(
                    x8t[:, 2 * c : min(2 * (c + 1), NU)],
                    x8_d[:, 2 * c : min(2 * (c + 1), NU)],
                )

            # fp8 head: the ENTIRE DR phase (all 16 tiles) only needs the
            # 6.5 MB fp8 stream. Tile 0's w8 is split in 4 so its first DR
            # only waits on ~100 KB; the DR wavefront paces with the
            # interleaved x8-chunk / w8-tile stream.
            wt8_0 = w8_pool.tile([128, NU, 2, 128], F8, tag="w8")
            w8ts[0] = wt8_0
            w8cuts = [0, 3, 6, 9, NU]
            for c in range(4):
                nc.sync.dma_start(
                    wt8_0[:, w8cuts[c] : w8cuts[c + 1]],
                    w8_d[0:128, w8cuts[c] : w8cuts[c + 1]],
                )
                fetch_x8(c)
            fetch_w8(1)
            fetch_x8(4)
            fetch_w8(2)
            fetch_x8(5)
            fetch_w8(3)
            for g in range(4, MT):
                fetch_w8(g)

            # --- eternal probs -> per-output bias (issued after the fp8
            # head; consumed from the first phase-2 epilogue ~45 us in) ---
            ang = small_pool.tile([128, GPAD * MT], F32)
            nc.sync.dma_start(ang[:], ang_d[:])
            cbt = small_pool.tile([128, MT], F32)
            nc.sync.dma_start(cbt[:], cbt_d[:])

            cosa = small_pool.tile([128, GPAD * MT], F32)
            # cos(a) = sin(a + pi/2); wrap into ACT Sin's [-pi, pi] domain
            # (|a| < 3pi/2 + pi holds for randn angles).
            nc.vector.add_range_wrap(
                cosa[:], ang[:], shift=math.pi / 2, bound=math.pi,
                period=2 * math.pi,
            )
            nc.scalar.activation(
                cosa[:], cosa[:], mybir.ActivationFunctionType.Sin
            )
            # tree-product over the 32 gate slots -> [128, MT]
            half = GPAD * MT // 2
            while half >= MT:
                nc.vector.tensor_mul(
                    cosa[:, 0:half], cosa[:, 0:half], cosa[:, half : 2 * half]
                )
                half //= 2
            bias_t = small_pool.tile([128, MT], F32)
            # probs = (prod cos)^2 / n
            nc.scalar.activation(
                bias_t[:],
                cosa[:, 0:MT],
                mybir.ActivationFunctionType.Square,
                scale=1.0 / math.sqrt(N),
            )
            nc.vector.tensor_add(bias_t[:], bias_t[:], cbt[:])

            # fp32r stream for phase 2: all xt slabs, then the 16 W tiles
            # (w_pool bufs=6 gates the 7th W fetch on tile-0-mains done;
            # nothing later in the ring is needed before that resolves)
            for s in range(KT):
                fetch_xt(s)
            for t in range(MT):
                fetch_w(t)

            # --- column-parallel GEMM over 16 m-tiles ---
            ot_box = [None]

            def epilogue(t, ps):
                j = t % MG
                if j == 0:
                    ot_box[0] = out_pool.tile([128, MG * BS], BF16, name="ot", tag="ot")
                ot = ot_box[0]
                nc.scalar.activation(
                    ot[:, j * BS : (j + 1) * BS],
                    ps[:],
                    mybir.ActivationFunctionType.Tanh,
                    bias=bias_t[:, t : t + 1],
                )
                g = t // MG
                if g == MT // MG - 1:
                    # final group: store each half as soon as its tanh is
                    # done so only a 256 KB store trails the last matmul
                    nc.scalar.dma_start(
                        out_d[g, :, j * BS : (j + 1) * BS],
                        ot[:, j * BS : (j + 1) * BS],
                    )
                elif j == MG - 1:
                    nc.scalar.dma_start(out_d[g], ot[:])

            def dr_mm(t, ps, u):
                """Wide fp8e5 DoubleRow correction matmul for one unit:
                rhs free 1024 -> all 512 out cols (hw accepts >512 moving
                for fp8). The u=0 DR opens the PSUM group (pending-zero
                covers the whole bank)."""
                first = u == 0
                nc.tensor.matmul(
                    ps[:],
                    lhsT=w8ts[t][:, u, :, :],
                    rhs=x8t[:, u, :, :],
                    start=first, stop=False,
                    perf_mode=DR,
                    skip_group_check=not first,
                )

            def main_mm(t, ps, kb):
                """fp32r main matmul for one k-tile; kb=KT-1 closes the
                accumulation group."""
                last = kb == KT - 1
                nc.tensor.matmul(
                    ps[:],
                    lhsT=wts[t][:, kb * 128 : (kb + 1) * 128],
                    rhs=xts[kb][:],
                    start=False, stop=last,
                    skip_group_check=not last,
                )

            # --- phase 1: ALL tiles' DR correction stages as one
            # anti-diagonal wavefront over (tile, x8-chunk), 8 PSUM banks
            # rotating; each tile's partial sum spills to SBUF on the
            # (otherwise idle) Vector engine, freeing its bank. The phase
            # is PE-bound: it consumes only the 6.5 MB fp8 stream while
            # the 21 MB fp32r stream lands behind it. ---
            ps1 = {}
            spills = {}
            drs_done = [0] * MT
            NCHX = (NU + 1) // 2  # 6 x8 chunks

            def dr_step(t, c):
                if drs_done[t] == 0:
                    ps1[t] = ps_pool.tile([128, BS], F32, tag="ps", bufs=8, name=f"ps1_{t}")
                for u in (2 * c, 2 * c + 1):
                    if u < NU:
                        nc.tensor.matmul(
                            ps1[t][:],
                            lhsT=w8ts[t][:, u, :, :],
                            rhs=x8t[:, u, :, :],
                            start=(u == 0), stop=(u == NU - 1),
                            perf_mode=DR,
                            skip_group_check=(u not in (0, NU - 1)),
                        )
                        drs_done[t] += 1
                if drs_done[t] == NU:
                    sp = spill_pool.tile([128, BS], F32, tag="sp", name=f"sp{t}")
                    nc.vector.tensor_copy(sp[:], ps1[t][:])
                    spills[t] = sp
                    w8ts.pop(t)

            # wavefront only while the fp8 stream is still arriving
            # (tiles 0..3); after that the stream runs ~2.5x ahead of the
            # PE, so per-tile blocks avoid the tile-hop wait-decode cost
            WF = 4
            for s in range(NCHX + WF - 1):
                for t in range(WF):
                    c = s - t
                    if 0 <= c < NCHX:
                        dr_step(t, c)
            for t in range(WF, MT):
                for c in range(NCHX):
                    dr_step(t, c)

            # --- phase 2: per tile, the 16 fp32r mains into a fresh bank;
            # the epilogue adds the spilled correction on DVE (in-place on
            # PSUM) and applies the fused tanh+bias on ACT as before. ---
            for t in range(MT):
                ps = ps_pool.tile([128, BS], F32, tag="ps", bufs=8, name=f"ps2_{t}")
                for kb in range(KT):
                    nc.tensor.matmul(
                        ps[:],
                        lhsT=wts[t][:, kb * 128 : (kb + 1) * 128],
                        rhs=xt_t[:, kb, :],
                        start=(kb == 0), stop=(kb == KT - 1),
                        skip_group_check=(kb not in (0, KT - 1)),
                    )
                nc.vector.tensor_add(ps[:], ps[:], spills[t][:])
                wts.pop(t)
                epilogue(t, ps)

    nc.compile()
    return nc


def to_fp32r(a):
    """Round fp32 -> fp32r storage (1-8-11 float in the top 20 bits, i.e.
    fp32 with the low 12 mantissa bits zeroed, round-to-nearest-even)."""
    u = np.ascontiguousarray(a, dtype=np.float32).view(np.uint32).astype(np.uint64)
    lsb = (u >> 12) & 1
    u = (u + 0x7FF + lsb) & 0xFFFFF000
    return u.astype(np.uint32).view(np.float32)


def _e5(a):
    return np.asarray(a, dtype=np.float32).astype(ml_dtypes.float8_e5m2)


def _relayout_w(w):
    """[N, M] -> w_dev[t*128 + p, kb*128 + m] = w[kb*128 + p, t*128 + m]
    so each m-tile's [128, N] slab is row-contiguous."""
    return w.reshape(KT, 128, MT, 128).transpose(2, 1, 0, 3).reshape(M, N)


def host_prep(x, eternal_weights, classical_weights, classical_biases):
    """Shard + lay out the inputs for the 8 cores (DMA-friendly layouts)."""
    x = np.ascontiguousarray(x, dtype=np.float32)
    w = np.ascontiguousarray(classical_weights, dtype=np.float32)
    cb = np.asarray(classical_biases, dtype=np.float32)

    xh = to_fp32r(x)
    wh = to_fp32r(w)
    w_dev = np.ascontiguousarray(_relayout_w(wh))

    # fp8 correction planes, packed per UNITS (see top of file)
    wh8 = _e5(wh)
    wl8 = _e5((w - wh).astype(np.float32))

    def _rk(a):  # [N, M] -> [MT, 128p, KT, 128m]
        return a.reshape(KT, 128, MT, 128).transpose(2, 1, 0, 3)

    rh, rl = _rk(wh8), _rk(wl8)
    w8u = np.empty((MT, 128, NU, 2, 128), dtype=wh8.dtype)
    for u, (kind, kb) in enumerate(UNITS):
        w8u[:, :, u, 0] = rh[:, :, kb]
        w8u[:, :, u, 1] = rl[:, :, kb] if kind == "hl" else rh[:, :, kb + 1]
    w8_dev = np.ascontiguousarray(w8u.reshape(M, NU, 2, 128))

    # angles actually used: [D, M, 3] -> [27, M]; device layout
    # ang[p, g*MT + t] = angle_g[t*128 + p], zero-padded to GPAD slots.
    a = np.transpose(np.asarray(eternal_weights[:, :M, :3], dtype=np.float32),
                     (0, 2, 1)).reshape(NGATE, M)
    ar = a.reshape(NGATE, MT, 128)  # [g, t, p]
    ang = np.zeros((128, GPAD, MT), dtype=np.float32)
    ang[:, :NGATE, :] = np.transpose(ar, (2, 0, 1))
    ang = np.ascontiguousarray(ang.reshape(128, GPAD * MT))

    cbt = np.ascontiguousarray(cb.reshape(MT, 128).T)  # [128, MT]

    def shard_xt(xs):
        # [BS, N] -> [128, KT, BS]: xt[p, kb, b] = xs[b, kb*128 + p]
        return xs.reshape(BS, KT, 128).transpose(2, 1, 0)

    in_maps = []
    for c in range(N_CORES):
        sl = slice(c * BS, (c + 1) * BS)
        xt = np.ascontiguousarray(shard_xt(xh[sl]).reshape(128, KT * BS))
        sl8 = shard_xt(_e5((x[sl] - xh[sl]).astype(np.float32)))  # [128, KT, BS]
        sh8 = shard_xt(_e5(xh[sl]))
        x8 = np.empty((128, NU, 2, BS), dtype=sl8.dtype)
        for u, (kind, kb) in enumerate(UNITS):
            x8[:, u, 0] = sl8[:, kb]
            x8[:, u, 1] = sh8[:, kb] if kind == "hl" else sl8[:, kb + 1]
        x8 = np.ascontiguousarray(x8)
        in_maps.append({
            "xt": xt, "x8": x8, "w": w_dev, "w8": w8_dev,
            "ang": ang, "cbt": cbt,
        })
    return in_maps


def host_post(results):
    """Reassemble [4096, 2048] from the 8 cores' out_dev blocks."""
    parts = []
    for c in range(N_CORES):
        od = np.asarray(results[c]["out_dev"]).astype(np.float32)
        # outT[(g*MG + j)*128 + ml, b] = od[g, ml, j*BS + b]
        outT = (
            od.reshape(MT // MG, 128, MG, BS)
            .transpose(0, 2, 1, 3)
            .reshape(M, BS)
        )
        parts.append(outT.T)  # [BS, M]
    return np.ascontiguousarray(np.concatenate(parts, axis=0), dtype=np.float32)


_NC_CACHE = {}


def _get_program():
    if "nc" not in _NC_CACHE:
        _NC_CACHE["nc"] = build_program()
    return _NC_CACHE["nc"]


def kernel(x, eternal_weights, eternal_biases, classical_weights, classical_biases,
           _trace=False):
    nc = _get_program()
    in_maps = host_prep(x, eternal_weights, classical_weights, classical_biases)
    res = run_bass_kernel_spmd(nc, in_maps, list(range(N_CORES)), trace=_trace)
    out = host_post(res.results)
    if _trace:
        kernel.last_exec_time_ns = res.exec_time_ns
        kernel.last_results = res
    return out


# revision 35
# speedup vs baseline: 1.1788x; 1.1788x over previous
"""Trainium2 Bass kernel for nn_EternalNeuralLayer.

Math: out = tanh(x @ W_c + b_c + probs[None, :]) where
probs[j] = |state[j, 0]|^2 after 27 nearest-neighbour circulant "gates"
applied to the uniform state 1/sqrt(n). Each gate matrix
G = cos*I - sin*P + sin*P^T is circulant, and the uniform vector is its
eigenvector with eigenvalue cos(theta), so the state stays uniform:
probs[j] = (prod_{d,g} cos(ew[d, j, g]))^2 / n   (g in 0..2, d in 0..8).

Sharding: data-parallel over the batch (8 cores x 512 rows). Every core
streams the full classical_weights [2048, 2048] and computes its
x-shard's GEMM as outT[m, b] = sum_k W[k, m] * xT[k, b] (output m on
partitions so the per-output bias (b_c + probs) is a per-partition ACT
bias), applies tanh on the Scalar engine directly out of PSUM, and
writes its outT shard. The eternal-probs product is computed on-device
per core from the [27, 2048] angle slice (tiny). No collectives.

GEMM precision: main pass xh @ Wh in float32r (fp32 with 11 explicit
mantissa bits, full PE rate, operands pre-rounded host-side).
Rounding-residual corrections run as wide fp8e5 (e5m2) DoubleRow
matmuls: one instruction computes two independent K=128 plane products
over all 512 out cols (rhs free 1024; the hw accepts >512 moving for
fp8) in 512 cycles -- 2x the fp32r MAC rate per k-tile (measured; the
cost model's 0.5 cyc/row does not materialize on TRN2 silicon in any
layout or perf mode). e5m2's 2^-15 dynamic range holds the
~2^-12-scale residuals unscaled, so corrections accumulate into the
SAME PSUM bank as the main pass (the first DR opens the group via the
2 KB pending-zero region; the last main closes it) and the epilogue
stays one fused tanh, stored as bf16 (tanh is in [-1,1]; bf16 adds
<= 2^-9; host upconverts). Correction coverage is partial (see UNITS):
x-residual on all 16 k-tiles, W-residual on 8. Measured absmax
1.52e-2 vs the 2e-2 gate (3.1e-3 at full coverage); the error is
fully deterministic -- the device result matches the host numpy model
of the exact rounding chain to 5 digits.

PE work per core: 16 m-tiles x (12 DoubleRow + 16 fp32r) matmuls x
512 cycles ~= 98 us at 2.4 GHz, vs 164 us for the previous 3-pass
fp32r hi/lo scheme.

Schedule: two global phases with a PSUM->SBUF spill between them.
Phase 1 runs ALL 16 tiles' fp8 DR correction stages (gated only on
the 6.5 MB fp8 stream -> PE-bound almost immediately); each tile's
partial sum is copied to SBUF by the otherwise-idle Vector engine,
freeing its PSUM bank (8 banks rotate over 32 accumulation groups).
Phase 2 runs each tile's 16 fp32r mains into a fresh bank -- by then
the whole 21 MB fp32r stream has landed behind phase 1, so every
phase-2 tile executes at the 3.63 us floor with zero gaps -- then DVE
adds the spilled correction in place on PSUM and ACT applies the
fused tanh+bias. The first 4 tiles of phase 1 are emitted as an
anti-diagonal wavefront over (tile, x8-chunk) to pace with the fp8
head; the rest run as straight per-tile blocks (tile-hopping costs
~23 ns/matmul in wait decode). All loads ride the single sync HWDGE
ring in consumption order (one ring saturates the ~343 GB/s core
bandwidth; FIFO order = precise arrival control). Output stores use
the scalar ring so a store waiting on ACT never head-of-line-blocks
loads.
"""

import math
import os
import sys

import numpy as np
import ml_dtypes

for _p in ("/opt/trn_rl_repo", "/root/.axon_site/_ro/trn_rl_repo"):
    if _p not in sys.path and os.path.isdir(_p):
        sys.path.append(_p)

import concourse.bass as bass  # noqa: E402
import concourse.tile as tile  # noqa: E402
from concourse import bacc, mybir  # noqa: E402
from concourse.bass_utils import run_bass_kernel_spmd  # noqa: E402

N_CORES = 8
B, N, M, D = 4096, 2048, 2048, 9
BS = B // N_CORES  # 512 batch rows per core
KT = N // 128  # 16 contraction tiles
MT = M // 128  # 16 output m-tiles
MG = 2  # m-tiles per output DMA group
NGATE = D * 3  # 27 rotation gates
GPAD = 32  # padded gate slots (pad with 0.0 -> cos = 1)

# Correction coverage. Each DoubleRow "unit" holds two K=128 plane
# products. ("hl", kb) pairs the x-residual with the W-residual for
# k-tile kb: Wh8[kb]*xl8[kb] + Wl8[kb]*xh8[kb]. ("xx", kb) packs two
# k-tiles of the x-residual correction only: Wh8[kb]*xl8[kb] +
# Wh8[kb+1]*xl8[kb+1]. With WCOV=8 the W-residual is corrected on
# 8/16 k-tiles: measured absmax 1.52e-2 vs the 2e-2 gate (vs 3.1e-3
# at full coverage), for 4 fewer 512-cycle PE instructions per m-tile.
# The error is fully deterministic (device result matches the host
# numpy model of the exact rounding chain to 5 digits).
WCOV = 8
UNITS = [("hl", kb) for kb in range(WCOV)] + [
    ("xx", kb) for kb in range(WCOV, KT, 2)
]
NU = len(UNITS)  # 12

F32 = mybir.dt.float32
F32R = mybir.dt.float32r
F8 = mybir.dt.float8e5
BF16 = mybir.dt.bfloat16
DR = mybir.MatmulPerfMode.DoubleRow


def build_program():
    nc = bacc.Bacc(
        "TRN2", target_bir_lowering=False, debug=False, num_devices=N_CORES
    )
    # xt[p, kb*BS + b] = xh[b, kb*128 + p]  (fp32r high part of x)
    xt_d = nc.dram_tensor("xt", [128, KT * BS], F32R, kind="ExternalInput").ap()
    # x8[p, u, pl, b]: correction-unit moving planes (see UNITS)
    x8_d = nc.dram_tensor("x8", [128, NU, 2, BS], F8, kind="ExternalInput").ap()
    # w[t*128 + p, kb*128 + m] = Wh[kb*128 + p, t*128 + m]  (fp32r)
    w_d = nc.dram_tensor("w", [M, N], F32R, kind="ExternalInput").ap()
    # w8[t*128+p, u, pl, m]: correction-unit stationary planes
    w8_d = nc.dram_tensor("w8", [M, NU, 2, 128], F8, kind="ExternalInput").ap()
    ang_d = nc.dram_tensor("ang", [128, GPAD * MT], F32, kind="ExternalInput").ap()
    cbt_d = nc.dram_tensor("cbt", [128, MT], F32, kind="ExternalInput").ap()
    # out_dev[g, ml, j*BS + b] = tanh(...)[m = (g*MG+j)*128 + ml, b]
    # bf16: tanh output is in [-1, 1], so bf16 adds <= 2^-9 abs error and
    # halves the store traffic; host_post upconverts to fp32.
    out_d = nc.dram_tensor(
        "out_dev", [MT // MG, 128, MG * BS], BF16, kind="ExternalOutput"
    ).ap()

    with tile.TileContext(nc) as tc:
        with (
            tc.tile_pool(name="xt", bufs=1) as xt_pool,
            tc.tile_pool(name="x8", bufs=1) as x8_pool,
            tc.tile_pool(name="w", bufs=6) as w_pool,
            tc.tile_pool(name="w8", bufs=MT) as w8_pool,
            tc.tile_pool(name="ps", bufs=8, space="PSUM") as ps_pool,
            tc.tile_pool(name="out", bufs=3) as out_pool,
            tc.tile_pool(name="spill", bufs=MT) as spill_pool,
            tc.tile_pool(name="small", bufs=1) as small_pool,
        ):
            # --- GEMM input DMAs, all on the sync HWDGE ring so arrival
            # ORDER is exactly the issue order (one ring saturates the
            # ~343 GB/s core bandwidth by itself). Head order feeds the
            # stream-split ramp: fp8 w8/x8 first (corrections run first),
            # then fp32r w/xt. Output stores use the scalar ring so a
            # store waiting on ACT never head-of-line-blocks loads. ---
            wts = {}
            w8ts = {}

            def fetch_w(t):
                wt = w_pool.tile([128, KT * 128], F32R, tag="w")
                nc.sync.dma_start(wt[:], w_d[t * 128 : (t + 1) * 128, :])
                wts[t] = wt

            def fetch_w8(t):
                wt8 = w8_pool.tile([128, NU, 2, 128], F8, tag="w8")
                nc.sync.dma_start(wt8[:], w8_d[t * 128 : (t + 1) * 128])
                w8ts[t] = wt8

            xts = []

            def fetch_xt(s):
                xtk = xt_pool.tile([128, BS], F32R, tag=f"xt{s}")
                nc.sync.dma_start(xtk[:], xt_d[:, s * BS : (s + 1) * BS])
                xts.append(xtk)

            # one fp8 x tile; fetched in 2-unit chunks
            NCH = (NU + 1) // 2  # 7 chunks of up to 2 units
            x8t = x8_pool.tile([128, NU, 2, BS], F8, name="x8t")

            def fetch_x8(c):
                nc.sync.dma_start(
                    x8t[:, 2 * c : min(2 * (c + 1), NU)],
                    x8_d[:, 2 * c : min(2 * (c + 1), NU)],
                )

            # fp8 head: the ENTIRE DR phase (all 16 tiles) only needs the
            # 6.5 MB fp8 stream. Tile 0's w8 is split in 4 so its first DR
            # only waits on ~100 KB; the DR wavefront paces with the
            # interleaved x8-chunk / w8-tile stream.
            wt8_0 = w8_pool.tile([128, NU, 2, 128], F8, tag="w8")
            w8ts[0] = wt8_0
            w8cuts = [0, 3, 6, 9, NU]
            for c in range(4):
                nc.sync.dma_start(
                    wt8_0[:, w8cuts[c] : w8cuts[c + 1]],
                    w8_d[0:128, w8cuts[c] : w8cuts[c + 1]],
                )
                fetch_x8(c)
            fetch_w8(1)
            fetch_x8(4)
            fetch_w8(2)
            fetch_x8(5)
            fetch_w8(3)
            for g in range(4, MT):
                fetch_w8(g)

            # --- eternal probs -> per-output bias (issued after the fp8
            # head; consumed from the first phase-2 epilogue ~45 us in) ---
            ang = small_pool.tile([128, GPAD * MT], F32)
            nc.sync.dma_start(ang[:], ang_d[:])
            cbt = small_pool.tile([128, MT], F32)
            nc.sync.dma_start(cbt[:], cbt_d[:])

            cosa = small_pool.tile([128, GPAD * MT], F32)
            # cos(a) = sin(a + pi/2); wrap into ACT Sin's [-pi, pi] domain
            # (|a| < 3pi/2 + pi holds for randn angles).
            nc.vector.add_range_wrap(
                cosa[:], ang[:], shift=math.pi / 2, bound=math.pi,
                period=2 * math.pi,
            )
            nc.scalar.activation(
                cosa[:], cosa[:], mybir.ActivationFunctionType.Sin
            )
            # tree-product over the 32 gate slots -> [128, MT]
            half = GPAD * MT // 2
            while half >= MT:
                nc.vector.tensor_mul(
                    cosa[:, 0:half], cosa[:, 0:half], cosa[:, half : 2 * half]
                )
                half //= 2
            bias_t = small_pool.tile([128, MT], F32)
            # probs = (prod cos)^2 / n
            nc.scalar.activation(
                bias_t[:],
                cosa[:, 0:MT],
                mybir.ActivationFunctionType.Square,
                scale=1.0 / math.sqrt(N),
            )
            nc.vector.tensor_add(bias_t[:], bias_t[:], cbt[:])

            # fp32r stream for phase 2: all xt slabs, then the 16 W tiles
            # (w_pool bufs=6 gates the 7th W fetch on tile-0-mains done;
            # nothing later in the ring is needed before that resolves)
            for s in range(KT):
                fetch_xt(s)
            for t in range(MT):
                fetch_w(t)

            # --- column-parallel GEMM over 16 m-tiles ---
            ot_box = [None]

            def epilogue(t, ps):
                j = t % MG
                if j == 0:
                    ot_box[0] = out_pool.tile([128, MG * BS], BF16, name="ot", tag="ot")
                ot = ot_box[0]
                nc.scalar.activation(
                    ot[:, j * BS : (j + 1) * BS],
                    ps[:],
                    mybir.ActivationFunctionType.Tanh,
                    bias=bias_t[:, t : t + 1],
                )
                g = t // MG
                if g == MT // MG - 1:
                    # final group: store each half as soon as its tanh is
                    # done so only a 256 KB store trails the last matmul
                    nc.scalar.dma_start(
                        out_d[g, :, j * BS : (j + 1) * BS],
                        ot[:, j * BS : (j + 1) * BS],
                    )
                elif j == MG - 1:
                    nc.scalar.dma_start(out_d[g], ot[:])

            def dr_mm(t, ps, u):
                """Wide fp8e5 DoubleRow correction matmul for one unit:
                rhs free 1024 -> all 512 out cols (hw accepts >512 moving
                for fp8). The u=0 DR opens the PSUM group (pending-zero
                covers the whole bank)."""
                first = u == 0
                nc.tensor.matmul(
                    ps[:],
                    lhsT=w8ts[t][:, u, :, :],
                    rhs=x8t[:, u, :, :],
                    start=first, stop=False,
                    perf_mode=DR,
                    skip_group_check=not first,
                )

            def main_mm(t, ps, kb):
                """fp32r main matmul for one k-tile; kb=KT-1 closes the
                accumulation group."""
                last = kb == KT - 1
                nc.tensor.matmul(
                    ps[:],
                    lhsT=wts[t][:, kb * 128 : (kb + 1) * 128],
                    rhs=xts[kb][:],
                    start=False, stop=last,
                    skip_group_check=not last,
                )

            # --- phase 1: ALL tiles' DR correction stages as one
            # anti-diagonal wavefront over (tile, x8-chunk), 8 PSUM banks
            # rotating; each tile's partial sum spills to SBUF on the
            # (otherwise idle) Vector engine, freeing its bank. The phase
            # is PE-bound: it consumes only the 6.5 MB fp8 stream while
            # the 21 MB fp32r stream lands behind it. ---
            ps1 = {}
            spills = {}
            drs_done = [0] * MT
            NCHX = (NU + 1) // 2  # 6 x8 chunks

            def dr_step(t, c):
                if drs_done[t] == 0:
                    ps1[t] = ps_pool.tile([128, BS], F32, tag="ps", bufs=8, name=f"ps1_{t}")
                for u in (2 * c, 2 * c + 1):
                    if u < NU:
                        nc.tensor.matmul(
                            ps1[t][:],
                            lhsT=w8ts[t][:, u, :, :],
                            rhs=x8t[:, u, :, :],
                            start=(u == 0), stop=(u == NU - 1),
                            perf_mode=DR,
                            skip_group_check=(u not in (0, NU - 1)),
                        )
                        drs_done[t] += 1
                if drs_done[t] == NU:
                    sp = spill_pool.tile([128, BS], F32, tag="sp", name=f"sp{t}")
                    nc.vector.tensor_copy(sp[:], ps1[t][:])
                    spills[t] = sp
                    w8ts.pop(t)

            # wavefront only while the fp8 stream is still arriving
            # (tiles 0..3); after that the stream runs ~2.5x ahead of the
            # PE, so per-tile blocks avoid the tile-hop wait-decode cost
            WF = 4
            for s in range(NCHX + WF - 1):
                for t in range(WF):
                    c = s - t
                    if 0 <= c < NCHX:
                        dr_step(t, c)
            for t in range(WF, MT):
                for c in range(NCHX):
                    dr_step(t, c)

            # --- phase 2: per tile, the 16 fp32r mains into a fresh bank;
            # the epilogue adds the spilled correction on DVE (in-place on
            # PSUM) and applies the fused tanh+bias on ACT as before. ---
            for t in range(MT):
                ps = ps_pool.tile([128, BS], F32, tag="ps", bufs=8, name=f"ps2_{t}")
                for kb in range(KT):
                    nc.tensor.matmul(
                        ps[:],
                        lhsT=wts[t][:, kb * 128 : (kb + 1) * 128],
                        rhs=xts[kb][:],
                        start=(kb == 0), stop=(kb == KT - 1),
                        skip_group_check=(kb not in (0, KT - 1)),
                    )
                nc.vector.tensor_add(ps[:], ps[:], spills[t][:])
                wts.pop(t)
                epilogue(t, ps)

    nc.compile()
    return nc


def to_fp32r(a):
    """Round fp32 -> fp32r storage (1-8-11 float in the top 20 bits, i.e.
    fp32 with the low 12 mantissa bits zeroed, round-to-nearest-even)."""
    u = np.ascontiguousarray(a, dtype=np.float32).view(np.uint32).astype(np.uint64)
    lsb = (u >> 12) & 1
    u = (u + 0x7FF + lsb) & 0xFFFFF000
    return u.astype(np.uint32).view(np.float32)


def _e5(a):
    return np.asarray(a, dtype=np.float32).astype(ml_dtypes.float8_e5m2)


def _relayout_w(w):
    """[N, M] -> w_dev[t*128 + p, kb*128 + m] = w[kb*128 + p, t*128 + m]
    so each m-tile's [128, N] slab is row-contiguous."""
    return w.reshape(KT, 128, MT, 128).transpose(2, 1, 0, 3).reshape(M, N)


def host_prep(x, eternal_weights, classical_weights, classical_biases):
    """Shard + lay out the inputs for the 8 cores (DMA-friendly layouts)."""
    x = np.ascontiguousarray(x, dtype=np.float32)
    w = np.ascontiguousarray(classical_weights, dtype=np.float32)
    cb = np.asarray(classical_biases, dtype=np.float32)

    xh = to_fp32r(x)
    wh = to_fp32r(w)
    w_dev = np.ascontiguousarray(_relayout_w(wh))

    # fp8 correction planes, packed per UNITS (see top of file)
    wh8 = _e5(wh)
    wl8 = _e5((w - wh).astype(np.float32))

    def _rk(a):  # [N, M] -> [MT, 128p, KT, 128m]
        return a.reshape(KT, 128, MT, 128).transpose(2, 1, 0, 3)

    rh, rl = _rk(wh8), _rk(wl8)
    w8u = np.empty((MT, 128, NU, 2, 128), dtype=wh8.dtype)
    for u, (kind, kb) in enumerate(UNITS):
        w8u[:, :, u, 0] = rh[:, :, kb]
        w8u[:, :, u, 1] = rl[:, :, kb] if kind == "hl" else rh[:, :, kb + 1]
    w8_dev = np.ascontiguousarray(w8u.reshape(M, NU, 2, 128))

    # angles actually used: [D, M, 3] -> [27, M]; device layout
    # ang[p, g*MT + t] = angle_g[t*128 + p], zero-padded to GPAD slots.
    a = np.transpose(np.asarray(eternal_weights[:, :M, :3], dtype=np.float32),
                     (0, 2, 1)).reshape(NGATE, M)
    ar = a.reshape(NGATE, MT, 128)  # [g, t, p]
    ang = np.zeros((128, GPAD, MT), dtype=np.float32)
    ang[:, :NGATE, :] = np.transpose(ar, (2, 0, 1))
    ang = np.ascontiguousarray(ang.reshape(128, GPAD * MT))

    cbt = np.ascontiguousarray(cb.reshape(MT, 128).T)  # [128, MT]

    def shard_xt(xs):
        # [BS, N] -> [128, KT, BS]: xt[p, kb, b] = xs[b, kb*128 + p]
        return xs.reshape(BS, KT, 128).transpose(2, 1, 0)

    in_maps = []
    for c in range(N_CORES):
        sl = slice(c * BS, (c + 1) * BS)
        xt = np.ascontiguousarray(shard_xt(xh[sl]).reshape(128, KT * BS))
        sl8 = shard_xt(_e5((x[sl] - xh[sl]).astype(np.float32)))  # [128, KT, BS]
        sh8 = shard_xt(_e5(xh[sl]))
        x8 = np.empty((128, NU, 2, BS), dtype=sl8.dtype)
        for u, (kind, kb) in enumerate(UNITS):
            x8[:, u, 0] = sl8[:, kb]
            x8[:, u, 1] = sh8[:, kb] if kind == "hl" else sl8[:, kb + 1]
        x8 = np.ascontiguousarray(x8)
        in_maps.append({
            "xt": xt, "x8": x8, "w": w_dev, "w8": w8_dev,
            "ang": ang, "cbt": cbt,
        })
    return in_maps


def host_post(results):
    """Reassemble [4096, 2048] from the 8 cores' out_dev blocks."""
    parts = []
    for c in range(N_CORES):
        od = np.asarray(results[c]["out_dev"]).astype(np.float32)
        # outT[(g*MG + j)*128 + ml, b] = od[g, ml, j*BS + b]
        outT = (
            od.reshape(MT // MG, 128, MG, BS)
            .transpose(0, 2, 1, 3)
            .reshape(M, BS)
        )
        parts.append(outT.T)  # [BS, M]
    return np.ascontiguousarray(np.concatenate(parts, axis=0), dtype=np.float32)


_NC_CACHE = {}


def _get_program():
    if "nc" not in _NC_CACHE:
        _NC_CACHE["nc"] = build_program()
    return _NC_CACHE["nc"]


def kernel(x, eternal_weights, eternal_biases, classical_weights, classical_biases,
           _trace=False):
    nc = _get_program()
    in_maps = host_prep(x, eternal_weights, classical_weights, classical_biases)
    res = run_bass_kernel_spmd(nc, in_maps, list(range(N_CORES)), trace=_trace)
    out = host_post(res.results)
    if _trace:
        kernel.last_exec_time_ns = res.exec_time_ns
        kernel.last_results = res
    return out
